# revision 33
# baseline (speedup 1.0000x reference)
"""Trainium2 Bass kernel for DynamicTokenMixing (16-head attention, N=4096, C=1024).

Sharding: head-parallel across 8 NeuronCores, 2 heads per core. Each core
computes q/k/v projections for its 2 heads, full attention for those heads,
and a partial output projection (row-parallel over Wproj); the host sums the
8 partials and adds the bias.

v1: all matmul operands in bf16 (PE native full rate; fp32r runs half rate
on real TRN2 for these shapes), stacked 128-deep output projection.

Per-core dataflow (all layouts chosen so no on-chip transposes of the
attention matrix are needed):
  qT, kT   = (x @ Wq_cols).T, (x @ Wkv_kcols).T     [dual-head stacked, 128 x 4096]
  vT       = (x @ Wkv_vcols).T  -> PE-transposed to v tiles [j, d] (+ ones col)
  ST[j,i]  = sum_d k[j,d] q[i,d]          (scores, transposed layout)
  ET       = exp(0.5 * ST)                (0.5 = gpd_ratio^2 * d^-0.5)
  AV^T     = sum_j v_ext[j,:]^T ET[j,:]   (row 64 = softmax denominator l[i])
  outT     = AV^T[0:64] * (1/l) (broadcast), both heads stacked [128, strip]
  out      = sum_strips outT.T @ Wp_rows  (partial; host adds across cores)
"""

import numpy as np

import concourse.bass as bass
import concourse.mybir as mybir
import concourse.tile as tile
from concourse import bacc
from concourse.bass_utils import run_bass_kernel_spmd
from concourse.masks import make_identity

F32 = mybir.dt.float32
F32R = mybir.dt.float32r
BF16 = mybir.dt.bfloat16

N = 4096          # tokens
C = 1024          # model dim
D = 64            # head dim
NHEADS = 16
GPD = 2
NCORES = 8
NJ = N // 128     # 32 key tiles
NCT = C // 128    # 8 contraction tiles
STRIP = 512       # query-strip width
NSTRIP = N // STRIP
JB = 2            # key tiles batched per exp instruction
SCORE_SCALE = GPD * GPD * (D ** -0.5)  # 0.5
# Schraudolph fast-exp on DVE: bf16(exp(SCORE_SCALE*s)) ~= bits(i16(s*A + B)).
# B tuned on the real score distribution (numpy study: rel err 1.3e-2).
FAST_A = SCORE_SCALE * 128.0 / float(np.log(2.0))   # 92.3324826
FAST_B = 16250.0
DVE_EXP_MOD = 3   # jp batches with jp % 3 == 2 use DVE fast-exp (1/3 of work);
                  # 1/2 overloads the in-order DVE (tail copies delay the
                  # WAR-critical exp) — measured 615us vs 504us at 1/3


def build_nc(repeat=1, hw_loop=False, body="all"):
    nc = bacc.Bacc("TRN2", target_bir_lowering=False, debug=False,
                   num_devices=NCORES)
    xT = nc.declare_dram_parameter("xT", [C, N], BF16, isOutput=False)
    wq = nc.declare_dram_parameter("wq", [C, 128], BF16, isOutput=False)
    wk = nc.declare_dram_parameter("wk", [C, 128], BF16, isOutput=False)
    wv = nc.declare_dram_parameter("wv", [C, 128], BF16, isOutput=False)
    wp = nc.declare_dram_parameter("wp", [128, C], BF16, isOutput=False)
    out = nc.declare_dram_parameter("out", [N, C], F32, isOutput=True)

    xT_r = xT[:].rearrange("(t p) n -> p t n", p=128)    # [128, 8, 4096]
    out_r = out[:].rearrange("(t p) o -> t p o", p=128)  # [32, 128, 1024]

    with tile.TileContext(nc) as tc:
        with (
            nc.allow_low_precision(reason="bf16 matmul inputs by design"),
            tc.tile_pool(name="persist", bufs=1) as persist,
            tc.tile_pool(name="small", bufs=4) as small,
        ):
            wq_sb = persist.tile([128, NCT, 128], BF16)
            wk_sb = persist.tile([128, NCT, 128], BF16)
            wv_sb = persist.tile([128, NCT, 128], BF16)
            wp_sb = persist.tile([128, C], BF16)
            # per-strip tiles so dependencies are fine-grained (phase overlap)
            qT_s = [persist.tile([128, STRIP], BF16, name=f"qT{i}")
                    for i in range(NSTRIP)]
            kT_s = [persist.tile([128, STRIP], BF16, name=f"kT{i}")
                    for i in range(NSTRIP)]
            vT_s = [persist.tile([128, STRIP], BF16, name=f"vT{i}")
                    for i in range(NSTRIP)]
            # per-key-tile v in natural layout: [j, (vA|1|vB|1)]
            vsb = [persist.tile([128, 130], BF16, name=f"vsb{j}")
                   for j in range(NJ)]
            # both heads' normalized outputs stacked: rows 0:64 h0, 64:128 h1
            outT = [persist.tile([128, STRIP], BF16, name=f"outT{i}")
                    for i in range(NSTRIP)]
            ident = persist.tile([128, 128], BF16)
            ones_f = persist.tile([128, D], F32)
            nc.gpsimd.memset(ones_f[:], 1.0)
            ones_t = persist.tile([65, D], F32R)
            nc.vector.tensor_copy(ones_t[:], ones_f[0:65, :])

            nc.sync.dma_start(wq_sb[:], wq[:].rearrange("(t p) m -> p t m", p=128))
            nc.sync.dma_start(wk_sb[:], wk[:].rearrange("(t p) m -> p t m", p=128))
            nc.sync.dma_start(wv_sb[:], wv[:].rearrange("(t p) m -> p t m", p=128))
            nc.sync.dma_start(wp_sb[:], wp[:])
            make_identity(nc, ident[:])
            for j in range(NJ):
                nc.vector.tensor_copy(vsb[j][:, 64:65], ones_f[:, 0:1])
                nc.vector.tensor_copy(vsb[j][:, 129:130], ones_f[:, 0:1])

            import contextlib

            def do_ph1():
                  with (
                      tc.tile_pool(name="ph1_sb", bufs=2) as ph1_sb,
                      tc.tile_pool(name="ph1_ps", bufs=2, space="PSUM") as ph1_ps,
                      tc.tile_pool(name="tp_ps", bufs=2, space="PSUM") as tp_ps,
                  ):
                      # ---- Phase 1: qT/kT/vT projections; vsb natural-layout tiles ----
                      for i in range(NSTRIP):
                          sl = bass.ts(i, STRIP)
                          xt = ph1_sb.tile([128, NCT, STRIP], BF16, tag="xt")
                          nc.sync.dma_start(xt[:], xT_r[:, :, sl])
                          q_ps = ph1_ps.tile([128, STRIP], F32, tag="q")
                          k_ps = ph1_ps.tile([128, STRIP], F32, tag="k")
                          v_ps = ph1_ps.tile([128, STRIP], F32, tag="v")
                          for c in range(NCT):
                              st, sp = (c == 0), (c == NCT - 1)
                              nc.tensor.matmul(q_ps[:], wq_sb[:, c, :], xt[:, c, :],
                                               start=st, stop=sp)
                              nc.tensor.matmul(k_ps[:], wk_sb[:, c, :], xt[:, c, :],
                                               start=st, stop=sp)
                              nc.tensor.matmul(v_ps[:], wv_sb[:, c, :], xt[:, c, :],
                                               start=st, stop=sp)
                          nc.vector.tensor_copy(qT_s[i][:], q_ps[:])
                          nc.vector.tensor_copy(kT_s[i][:], k_ps[:])
                          nc.vector.tensor_copy(vT_s[i][:], v_ps[:])
                          for jj in range(STRIP // 128):
                              j = i * (STRIP // 128) + jj
                              tp = tp_ps.tile([128, 128], BF16, tag="tp")
                              nc.tensor.transpose(tp[:], vT_s[i][:, bass.ts(jj, 128)],
                                                  ident[:])
                              nc.vector.tensor_copy(vsb[j][:, 0:64], tp[:, 0:64])
                              nc.vector.tensor_copy(vsb[j][:, 65:129], tp[:, 64:128])

            def do_ph2():
                  # ---- Phase 2+3: attention + projection, pipelined per strip ----
                  with (
                      tc.tile_pool(name="att_et", bufs=4) as et_pool,
                      tc.tile_pool(name="pr_sb", bufs=2) as pr_sb,
                      tc.tile_pool(name="att_st", bufs=1, space="PSUM") as st_pool,
                      tc.tile_pool(name="att_av", bufs=1, space="PSUM") as av_pool,
                      tc.tile_pool(name="att_bc", bufs=1, space="PSUM") as bc_pool,
                      tc.tile_pool(name="pr_ps", bufs=1, space="PSUM") as pr_ps,
                  ):
                      heads = ((0, slice(0, 64)), (1, slice(64, 128)))

                      def emit_tail(i, av):
                          # normalize strip i (reads av, frees it) + project
                          for h, _ in heads:
                              stage = small.tile([65, STRIP], F32, tag="stage")
                              nc.vector.tensor_copy(stage[:], av[h][:])
                              rec_r = small.tile([65, STRIP], F32R, tag="rec_r")
                              nc.vector.reciprocal(rec_r[64:65, :], stage[64:65, :])
                              bc = bc_pool.tile([64, STRIP], F32, tag="bc")
                              nc.tensor.matmul(bc[:], ones_t[64:65, :],
                                               rec_r[64:65, :], start=True, stop=True)
                              nc.vector.tensor_mul(outT[i][h * 64:h * 64 + 64, :],
                                                   stage[0:64, :], bc[:])
                          for t in range(STRIP // 128):
                              it = i * (STRIP // 128) + t
                              tsl = bass.ts(t, 128)
                              ob = pr_sb.tile([128, C], F32, tag="ob")
                              for oc in range(C // STRIP):
                                  osl = bass.ts(oc, STRIP)
                                  pp = pr_ps.tile([128, STRIP], F32, tag="pp")
                                  nc.tensor.matmul(pp[:], outT[i][:, tsl],
                                                   wp_sb[:, osl], start=True, stop=True)
                                  nc.vector.tensor_copy(ob[:, osl], pp[:])
                              nc.sync.dma_start(out_r[it], ob[:])

                      def emit_avs(av, jp, ets):
                          for h, _ in heads:
                              for u in range(JB):
                                  j = JB * jp + u
                                  nc.tensor.matmul(
                                      av[h][:],
                                      vsb[j][:, h * 65:h * 65 + 65],
                                      ets[h][:, bass.ts(u, STRIP)],
                                      start=(j == 0), stop=(j == NJ - 1),
                                      skip_group_check=True,
                                  )

                      prev = None  # (i, av) of the strip awaiting its tail
                      for i in range(NSTRIP):
                          av = {h: av_pool.tile([65, STRIP], F32, tag=f"av{h}",
                                                name=f"av{h}")
                                for h, _ in heads}
                          for jp in range(NJ // JB):
                              # both heads' scores first: keeps PE fed while
                              # Act exps the previous batch (in-order engines)
                              sts = {}
                              for h, hs in heads:
                                  st = st_pool.tile([128, JB * STRIP], F32,
                                                    tag=f"st{h}")
                                  for u in range(JB):
                                      j = JB * jp + u
                                      nc.tensor.matmul(
                                          st[:, bass.ts(u, STRIP)],
                                          kT_s[j // (STRIP // 128)][hs, bass.ts(
                                              j % (STRIP // 128), 128)],
                                          qT_s[i][hs, :],
                                          start=True, stop=True,
                                      )
                                  sts[h] = st
                              ets = {}
                              for h, _ in heads:
                                  et = et_pool.tile([128, JB * STRIP], BF16,
                                                    tag=f"et{h}")
                                  if h == 1:
                                      # head 1's exps all on DVE (fast-exp):
                                      # each head gets an independent exp
                                      # pipeline on its own engine
                                      nc.vector.tensor_scalar(
                                          et[:].bitcast(mybir.dt.int16),
                                          sts[h][:], FAST_A, FAST_B,
                                          mybir.AluOpType.mult,
                                          mybir.AluOpType.add,
                                      )
                                  else:
                                      nc.scalar.activation(
                                          et[:], sts[h][:],
                                          mybir.ActivationFunctionType.Exp,
                                          scale=SCORE_SCALE,
                                      )
                                  ets[h] = et
                              emit_avs(av, jp, ets)
                              if jp == 2 and prev is not None:
                                  # previous strip's tail, interleaved so the
                                  # Act engine isn't starved during it
                                  emit_tail(*prev)
                                  prev = None
                          prev = (i, av)
                      emit_tail(*prev)

            if body == "ph2":
                do_ph1()
            rep_iter = ([None] if hw_loop and repeat > 1 else range(repeat))
            for _rep in rep_iter:
                with (tc.For_i(0, repeat, 1) if hw_loop and repeat > 1
                      else contextlib.nullcontext()):
                    if body in ("all", "ph1"):
                        do_ph1()
                    if body in ("all", "ph2"):
                        do_ph2()
    nc.finalize()
    return nc


def _colk(h):
    base = h * D if h < 8 else 2 * 512 + (h - 8) * D
    return slice(base, base + D)


def _colv(h):
    base = 512 + h * D if h < 8 else 3 * 512 + (h - 8) * D
    return slice(base, base + D)


def make_in_maps(x, Wq, Wkv, Wproj):
    import ml_dtypes
    bf16 = ml_dtypes.bfloat16
    x = np.asarray(x, np.float32).reshape(N, C)
    Wq = np.asarray(Wq, np.float32)
    Wkv = np.asarray(Wkv, np.float32)
    Wproj = np.asarray(Wproj, np.float32)
    xT = np.ascontiguousarray(x.T).astype(bf16)
    in_maps = []
    for core in range(NCORES):
        h0, h1 = 2 * core, 2 * core + 1
        in_maps.append({
            "xT": xT,
            "wq": np.ascontiguousarray(
                np.concatenate([Wq[:, h0 * D:(h0 + 1) * D],
                                Wq[:, h1 * D:(h1 + 1) * D]], axis=1)).astype(bf16),
            "wk": np.ascontiguousarray(
                np.concatenate([Wkv[:, _colk(h0)], Wkv[:, _colk(h1)]],
                               axis=1)).astype(bf16),
            "wv": np.ascontiguousarray(
                np.concatenate([Wkv[:, _colv(h0)], Wkv[:, _colv(h1)]],
                               axis=1)).astype(bf16),
            "wp": np.ascontiguousarray(
                np.concatenate([Wproj[h0 * D:(h0 + 1) * D, :],
                                Wproj[h1 * D:(h1 + 1) * D, :]],
                               axis=0)).astype(bf16),
        })
    return in_maps


_NC = None


def _get_nc():
    global _NC
    if _NC is None:
        _NC = build_nc()
    return _NC


def run_spmd(in_maps, **kwargs):
    return run_bass_kernel_spmd(_get_nc(), in_maps, list(range(NCORES)), **kwargs)


def kernel(x, Wq, Wkv, Wproj, bproj, H=None, W=None, **_unused):
    in_maps = make_in_maps(x, Wq, Wkv, Wproj)
    res = run_spmd(in_maps)
    acc = np.zeros((N, C), np.float64)
    for r in res.results:
        acc += r["out"]
    out = acc.astype(np.float32) + np.asarray(bproj, np.float32)[None, :]
    return out.reshape(1, N, C)


if __name__ == "__main__":
    nc = build_nc()
    print("built ok")


# revision 42
# speedup vs baseline: 1.0767x; 1.0767x over previous
"""Trainium2 Bass kernel for DynamicTokenMixing (16-head attention, N=4096, C=1024).

Sharding: head-parallel across 8 NeuronCores, 2 heads per core. Each core
computes q/k/v projections for its 2 heads, full attention for those heads,
and a partial output projection (row-parallel over Wproj); the host sums the
8 partials and adds the bias.

v1: all matmul operands in bf16 (PE native full rate; fp32r runs half rate
on real TRN2 for these shapes), stacked 128-deep output projection.

Per-core dataflow (all layouts chosen so no on-chip transposes of the
attention matrix are needed):
  qT, kT   = (x @ Wq_cols).T, (x @ Wkv_kcols).T     [dual-head stacked, 128 x 4096]
  vT       = (x @ Wkv_vcols).T  -> PE-transposed to v tiles [j, d] (+ ones col)
  ST[j,i]  = sum_d k[j,d] q[i,d]          (scores, transposed layout)
  ET       = exp(0.5 * ST)                (0.5 = gpd_ratio^2 * d^-0.5)
  AV^T     = sum_j v_ext[j,:]^T ET[j,:]   (row 64 = softmax denominator l[i])
  outT     = AV^T[0:64] * (1/l) (broadcast), both heads stacked [128, strip]
  out      = sum_strips outT.T @ Wp_rows  (partial; host adds across cores)
"""

import numpy as np

import concourse.bass as bass
import concourse.mybir as mybir
import concourse.tile as tile
from concourse import bacc
from concourse.bass_utils import run_bass_kernel_spmd
from concourse.masks import make_identity

F32 = mybir.dt.float32
F32R = mybir.dt.float32r
BF16 = mybir.dt.bfloat16

N = 4096          # tokens
C = 1024          # model dim
D = 64            # head dim
NHEADS = 16
GPD = 2
NCORES = 8
NJ = N // 128     # 32 key tiles
NCT = C // 128    # 8 contraction tiles
STRIP = 512       # query-strip width
NSTRIP = N // STRIP
JB = 2            # key tiles batched per exp instruction
SCORE_SCALE = GPD * GPD * (D ** -0.5)  # 0.5
# Schraudolph fast-exp on DVE: bf16(exp(SCORE_SCALE*s)) ~= bits(i16(s*A + B)).
# B tuned on the real score distribution (numpy study: rel err 1.3e-2).
FAST_A = SCORE_SCALE * 128.0 / float(np.log(2.0))   # 92.3324826
FAST_B = 16250.0
# Exp engine split is BY HEAD: head 0 -> Act (exact exp), head 1 -> DVE
# (fast-exp). Each head's score-buffer WAR then waits on its own engine,
# so the two head pipelines pace independently (-67us/iter vs jp-rotation).


def build_nc(repeat=1, hw_loop=False, body="all"):
    nc = bacc.Bacc("TRN2", target_bir_lowering=False, debug=False,
                   num_devices=NCORES)
    xT = nc.declare_dram_parameter("xT", [C, N], BF16, isOutput=False)
    wq = nc.declare_dram_parameter("wq", [C, 128], BF16, isOutput=False)
    wk = nc.declare_dram_parameter("wk", [C, 128], BF16, isOutput=False)
    wv = nc.declare_dram_parameter("wv", [C, 128], BF16, isOutput=False)
    wp = nc.declare_dram_parameter("wp", [128, C], BF16, isOutput=False)
    out = nc.declare_dram_parameter("out", [N, C], F32, isOutput=True)

    xT_r = xT[:].rearrange("(t p) n -> p t n", p=128)    # [128, 8, 4096]
    out_r = out[:].rearrange("(t p) o -> t p o", p=128)  # [32, 128, 1024]

    with tile.TileContext(nc) as tc:
        with (
            nc.allow_low_precision(reason="bf16 matmul inputs by design"),
            tc.tile_pool(name="persist", bufs=1) as persist,
            tc.tile_pool(name="small", bufs=4) as small,
        ):
            wq_sb = persist.tile([128, NCT, 128], BF16)
            wk_sb = persist.tile([128, NCT, 128], BF16)
            wv_sb = persist.tile([128, NCT, 128], BF16)
            wp_sb = persist.tile([128, C], BF16)
            # per-strip tiles so dependencies are fine-grained (phase overlap)
            qT_s = [persist.tile([128, STRIP], BF16, name=f"qT{i}")
                    for i in range(NSTRIP)]
            kT_s = [persist.tile([128, STRIP], BF16, name=f"kT{i}")
                    for i in range(NSTRIP)]
            vT_s = [persist.tile([128, STRIP], BF16, name=f"vT{i}")
                    for i in range(NSTRIP)]
            # per-key-tile v in natural layout: [j, (vA|1|vB|1)]
            vsb = [persist.tile([128, 130], BF16, name=f"vsb{j}")
                   for j in range(NJ)]
            # both heads' normalized outputs stacked: rows 0:64 h0, 64:128 h1
            outT = [persist.tile([128, STRIP], BF16, name=f"outT{i}")
                    for i in range(NSTRIP)]
            ident = persist.tile([128, 128], BF16)
            ones_f = persist.tile([128, D], F32)
            nc.gpsimd.memset(ones_f[:], 1.0)
            ones_t = persist.tile([65, D], F32R)
            nc.vector.tensor_copy(ones_t[:], ones_f[0:65, :])

            nc.sync.dma_start(wq_sb[:], wq[:].rearrange("(t p) m -> p t m", p=128))
            nc.sync.dma_start(wk_sb[:], wk[:].rearrange("(t p) m -> p t m", p=128))
            nc.sync.dma_start(wv_sb[:], wv[:].rearrange("(t p) m -> p t m", p=128))
            nc.sync.dma_start(wp_sb[:], wp[:])
            make_identity(nc, ident[:])
            for j in range(NJ):
                nc.vector.tensor_copy(vsb[j][:, 64:65], ones_f[:, 0:1])
                nc.vector.tensor_copy(vsb[j][:, 129:130], ones_f[:, 0:1])

            import contextlib

            def do_ph1():
                  with (
                      tc.tile_pool(name="ph1_sb", bufs=2) as ph1_sb,
                      tc.tile_pool(name="ph1_ps", bufs=2, space="PSUM") as ph1_ps,
                      tc.tile_pool(name="tp_ps", bufs=2, space="PSUM") as tp_ps,
                  ):
                      # ---- Phase 1: qT/kT/vT projections; vsb natural-layout tiles ----
                      for i in range(NSTRIP):
                          sl = bass.ts(i, STRIP)
                          xt = ph1_sb.tile([128, NCT, STRIP], BF16, tag="xt")
                          nc.sync.dma_start(xt[:], xT_r[:, :, sl])
                          q_ps = ph1_ps.tile([128, STRIP], F32, tag="q")
                          k_ps = ph1_ps.tile([128, STRIP], F32, tag="k")
                          v_ps = ph1_ps.tile([128, STRIP], F32, tag="v")
                          for c in range(NCT):
                              st, sp = (c == 0), (c == NCT - 1)
                              nc.tensor.matmul(q_ps[:], wq_sb[:, c, :], xt[:, c, :],
                                               start=st, stop=sp)
                              nc.tensor.matmul(k_ps[:], wk_sb[:, c, :], xt[:, c, :],
                                               start=st, stop=sp)
                              nc.tensor.matmul(v_ps[:], wv_sb[:, c, :], xt[:, c, :],
                                               start=st, stop=sp)
                          nc.vector.tensor_copy(qT_s[i][:], q_ps[:])
                          nc.vector.tensor_copy(kT_s[i][:], k_ps[:])
                          nc.vector.tensor_copy(vT_s[i][:], v_ps[:])
                          for jj in range(STRIP // 128):
                              j = i * (STRIP // 128) + jj
                              tp = tp_ps.tile([128, 128], BF16, tag="tp")
                              nc.tensor.transpose(tp[:], vT_s[i][:, bass.ts(jj, 128)],
                                                  ident[:])
                              nc.vector.tensor_copy(vsb[j][:, 0:64], tp[:, 0:64])
                              nc.vector.tensor_copy(vsb[j][:, 65:129], tp[:, 64:128])

            def do_ph2():
                  # ---- Phase 2+3: attention + projection, pipelined per strip ----
                  with (
                      tc.tile_pool(name="att_et", bufs=4) as et_pool,
                      tc.tile_pool(name="pr_sb", bufs=2) as pr_sb,
                      tc.tile_pool(name="att_st", bufs=1, space="PSUM") as st_pool,
                      tc.tile_pool(name="att_av", bufs=1, space="PSUM") as av_pool,
                      tc.tile_pool(name="att_bc", bufs=1, space="PSUM") as bc_pool,
                      tc.tile_pool(name="pr_ps", bufs=1, space="PSUM") as pr_ps,
                  ):
                      heads = ((0, slice(0, 64)), (1, slice(64, 128)))

                      def emit_tail(i, av):
                          # normalize strip i (reads av, frees it) + project
                          for h, _ in heads:
                              stage = small.tile([65, STRIP], F32, tag="stage")
                              nc.vector.tensor_copy(stage[:], av[h][:])
                              rec_r = small.tile([65, STRIP], F32R, tag="rec_r")
                              nc.vector.reciprocal(rec_r[64:65, :], stage[64:65, :])
                              bc = bc_pool.tile([64, STRIP], F32, tag="bc")
                              nc.tensor.matmul(bc[:], ones_t[64:65, :],
                                               rec_r[64:65, :], start=True, stop=True)
                              nc.vector.tensor_mul(outT[i][h * 64:h * 64 + 64, :],
                                                   stage[0:64, :], bc[:])
                          for t in range(STRIP // 128):
                              it = i * (STRIP // 128) + t
                              tsl = bass.ts(t, 128)
                              ob = pr_sb.tile([128, C], F32, tag="ob")
                              for oc in range(C // STRIP):
                                  osl = bass.ts(oc, STRIP)
                                  pp = pr_ps.tile([128, STRIP], F32, tag="pp")
                                  nc.tensor.matmul(pp[:], outT[i][:, tsl],
                                                   wp_sb[:, osl], start=True, stop=True)
                                  nc.vector.tensor_copy(ob[:, osl], pp[:])
                              nc.sync.dma_start(out_r[it], ob[:])

                      def emit_avs(av, jp, ets):
                          for h, _ in heads:
                              for u in range(JB):
                                  j = JB * jp + u
                                  nc.tensor.matmul(
                                      av[h][:],
                                      vsb[j][:, h * 65:h * 65 + 65],
                                      ets[h][:, bass.ts(u, STRIP)],
                                      start=(j == 0), stop=(j == NJ - 1),
                                      skip_group_check=True,
                                  )

                      prev = None  # (i, av) of the strip awaiting its tail
                      for i in range(NSTRIP):
                          av = {h: av_pool.tile([65, STRIP], F32, tag=f"av{h}",
                                                name=f"av{h}")
                                for h, _ in heads}
                          for jp in range(NJ // JB):
                              # both heads' scores first: keeps PE fed while
                              # Act exps the previous batch (in-order engines)
                              sts = {}
                              for h, hs in heads:
                                  st = st_pool.tile([128, JB * STRIP], F32,
                                                    tag=f"st{h}")
                                  for u in range(JB):
                                      j = JB * jp + u
                                      nc.tensor.matmul(
                                          st[:, bass.ts(u, STRIP)],
                                          kT_s[j // (STRIP // 128)][hs, bass.ts(
                                              j % (STRIP // 128), 128)],
                                          qT_s[i][hs, :],
                                          start=True, stop=True,
                                      )
                                  sts[h] = st
                              ets = {}
                              for h, _ in heads:
                                  et = et_pool.tile([128, JB * STRIP], BF16,
                                                    tag=f"et{h}")
                                  if h == 1:
                                      # head 1's exps all on DVE (fast-exp):
                                      # each head gets an independent exp
                                      # pipeline on its own engine
                                      nc.vector.tensor_scalar(
                                          et[:].bitcast(mybir.dt.int16),
                                          sts[h][:], FAST_A, FAST_B,
                                          mybir.AluOpType.mult,
                                          mybir.AluOpType.add,
                                      )
                                  else:
                                      nc.scalar.activation(
                                          et[:], sts[h][:],
                                          mybir.ActivationFunctionType.Exp,
                                          scale=SCORE_SCALE,
                                      )
                                  ets[h] = et
                              emit_avs(av, jp, ets)
                              if jp == 2 and prev is not None:
                                  # previous strip's tail, interleaved so the
                                  # Act engine isn't starved during it
                                  emit_tail(*prev)
                                  prev = None
                          prev = (i, av)
                      emit_tail(*prev)

            if body == "ph2":
                do_ph1()
            rep_iter = ([None] if hw_loop and repeat > 1 else range(repeat))
            for _rep in rep_iter:
                with (tc.For_i(0, repeat, 1) if hw_loop and repeat > 1
                      else contextlib.nullcontext()):
                    if body in ("all", "ph1"):
                        do_ph1()
                    if body in ("all", "ph2"):
                        do_ph2()
    nc.finalize()
    return nc


def _colk(h):
    base = h * D if h < 8 else 2 * 512 + (h - 8) * D
    return slice(base, base + D)


def _colv(h):
    base = 512 + h * D if h < 8 else 3 * 512 + (h - 8) * D
    return slice(base, base + D)


def make_in_maps(x, Wq, Wkv, Wproj):
    import ml_dtypes
    bf16 = ml_dtypes.bfloat16
    x = np.asarray(x, np.float32).reshape(N, C)
    Wq = np.asarray(Wq, np.float32)
    Wkv = np.asarray(Wkv, np.float32)
    Wproj = np.asarray(Wproj, np.float32)
    xT = np.ascontiguousarray(x.T).astype(bf16)
    in_maps = []
    for core in range(NCORES):
        h0, h1 = 2 * core, 2 * core + 1
        in_maps.append({
            "xT": xT,
            "wq": np.ascontiguousarray(
                np.concatenate([Wq[:, h0 * D:(h0 + 1) * D],
                                Wq[:, h1 * D:(h1 + 1) * D]], axis=1)).astype(bf16),
            "wk": np.ascontiguousarray(
                np.concatenate([Wkv[:, _colk(h0)], Wkv[:, _colk(h1)]],
                               axis=1)).astype(bf16),
            "wv": np.ascontiguousarray(
                np.concatenate([Wkv[:, _colv(h0)], Wkv[:, _colv(h1)]],
                               axis=1)).astype(bf16),
            "wp": np.ascontiguousarray(
                np.concatenate([Wproj[h0 * D:(h0 + 1) * D, :],
                                Wproj[h1 * D:(h1 + 1) * D, :]],
                               axis=0)).astype(bf16),
        })
    return in_maps


_NC = None


def _get_nc():
    global _NC
    if _NC is None:
        _NC = build_nc()
    return _NC


def run_spmd(in_maps, **kwargs):
    return run_bass_kernel_spmd(_get_nc(), in_maps, list(range(NCORES)), **kwargs)


def kernel(x, Wq, Wkv, Wproj, bproj, H=None, W=None, **_unused):
    in_maps = make_in_maps(x, Wq, Wkv, Wproj)
    res = run_spmd(in_maps)
    acc = np.zeros((N, C), np.float64)
    for r in res.results:
        acc += r["out"]
    out = acc.astype(np.float32) + np.asarray(bproj, np.float32)[None, :]
    return out.reshape(1, N, C)


if __name__ == "__main__":
    nc = build_nc()
    print("built ok")
